# revision 58
# baseline (speedup 1.0000x reference)
"""Trainium2 Bass kernel for nn_MemorySystem (cosine-sim attention memory read).

reference:
    x_norm = ||x||_row (B,1); m_norm = ||m||_row (S,1)
    sims = (x @ m^T) / max(x_norm * m_norm^T, 1e-8)
    attn = softmax(8.0 * sims, axis=1)
    out  = attn @ m                       # (B, D)

Sharding: memory_bank rows split across 8 NeuronCores (8192 rows each).
Each core computes, for its shard, the un-normalized softmax numerator
O_c = exp(S_c) @ m_c (B, D) and denominator Z_c = sum_s exp (B,), using the
bounded-score property (|8*cos| <= 8) to skip the max-subtraction entirely.
Per 512-query pass, the bf16 [qp, 513] partials (O|Z) are exchanged with a
mesh AllToAll (each core receives the 8 per-core partials for its own query
slice) and summed locally in f32 on the DVE; each core divides O/Z and
emits its rows.  The host reassembles the slices per CHUNKS.

Optimizations vs the original ReduceScatter baseline (~317-405us measured;
now ~330 under thermal throttle, ~300 clean):
  - AllToAll (mesh alg, pure data movement) replaces ReduceScatter (RDH,
    CC-core reduce-bound: 91us -> ~10-30us per collective); the 8-way
    add moves to a cheap local DVE chain, which also upgrades the
    reduction to f32 (rel err 8.9e-3 -> 7.9e-3; gate 2e-2).
  - One A2A per pass (chunking the last pass was tried and lost: under
    the PE keep-alive each collective's ~5us fixed cost dominates).
  - mm2 (exp @ m) runs in fp8e4m3 DoubleRow; its single-matmul thunks are
    interleaved between mm1's chunks so mm2's 154ns LDWEIGHTS hides under
    mm1's 213ns streams in the PE weight shadow buffer.
  - Z-accumulation (zadd) runs on the otherwise-idle GpSimd engine; the
    zsum/zrow/ztp chain is emitted before the final mm2 drain to overlap.
  - x prep is a per-tile DMA->square->rsqrt->finish pipeline emitted ahead
    of the m-prime, putting mm1(0) at ~8us instead of ~22us.
  - PE keep-alive: ~130 dependency-free dummy matmuls after the last pass
    plus a fin8-gated block of 45 (waits for the last A2A, covers the
    divide chain) hold the HAM activity state up through the tail; without
    them the HAM drops the core to a 50% utilization cap the moment
    compute drains, halving the exposed A2A + divide chain (A2As measured
    2-3x faster with the keep-alive in place).
  - post-collective divide chains are emitted AFTER all compute
    (tile_wait_until) so the strict-FIFO engine queues can never stall
    mid-kernel on a collective dependency.
"""

import sys

sys.path.insert(0, "/opt/trn_rl_repo")

import numpy as np
from contextlib import ExitStack

B, S, D = 1024, 65536, 512
NCORES = 8
S_SHARD = S // NCORES  # 8192
P = 128

ST = S_SHARD // P  # 64 s-tiles per core
QT = B // P  # 8 q-tiles
DC = D // P  # 4 d-chunks
# uneven passes: big first pass overlaps the m-load, small last pass keeps
# the final (unoverlapped) ReduceScatter payload small
PASSES = [(0, 512), (512, 512)]  # (q start, q count)
CHUNKS = [(0, 512), (512, 512)]
QP = 512  # max pass width (tile allocation size)
QPT = QP // P  # 4 q-tiles max per pass
NPAIR = ST // 2  # 32 s-tile pairs (DoubleRow mm2 granularity)
LAG = 10  # load runs this many s-tiles ahead of pass-0 compute
MB = 4  # m rows DMA'd per batched load (tiles per dma_start)

MAGIC = 0x5F3759DF

_CACHE = {}


def _build():
    import concourse.bass as bass
    import concourse.tile as tile
    from concourse import bacc, mybir
    from concourse.masks import make_identity

    f32 = mybir.dt.float32
    bf16 = mybir.dt.bfloat16
    f8 = mybir.dt.float8e4
    u32 = mybir.dt.uint32
    AF = mybir.ActivationFunctionType
    ALU = mybir.AluOpType
    DR = mybir.MatmulPerfMode.DoubleRow

    nc = bacc.Bacc(None, num_devices=NCORES)
    x_ext = nc.declare_dram_parameter("x", [B, D], f32, isOutput=False)
    m_ext = nc.declare_dram_parameter("mem", [S_SHARD, D], f32, isOutput=False)
    out_ext = nc.declare_dram_parameter("out", [B // NCORES, D], f32, isOutput=True)

    with tile.TileContext(nc) as tc, ExitStack() as ctx:
        persist = ctx.enter_context(tc.tile_pool(name="persist", bufs=1))
        xfp = ctx.enter_context(tc.tile_pool(name="xfp", bufs=1))
        loadp = ctx.enter_context(tc.tile_pool(name="load", bufs=3))
        mbp = ctx.enter_context(tc.tile_pool(name="mbp", bufs=8))
        sqp = ctx.enter_context(tc.tile_pool(name="sqp", bufs=2))
        work = ctx.enter_context(tc.tile_pool(name="work", bufs=2))
        pt8p = ctx.enter_context(tc.tile_pool(name="pt8p", bufs=3))
        zp = ctx.enter_context(tc.tile_pool(name="zp", bufs=2))
        stp = ctx.enter_context(tc.tile_pool(name="stp", bufs=2))
        finp = ctx.enter_context(tc.tile_pool(name="finp", bufs=2))
        dram = ctx.enter_context(tc.tile_pool(name="dram", bufs=4, space="DRAM"))
        # PSUM: 8 banks total. sc(2) + o2(QPT=4) + tp(2) = 8
        psum_sc = ctx.enter_context(tc.tile_pool(name="psc", bufs=2, space="PSUM"))
        psum_o = ctx.enter_context(tc.tile_pool(name="po", bufs=QPT, space="PSUM"))
        psum_tp = ctx.enter_context(tc.tile_pool(name="ptp", bufs=2, space="PSUM"))

        # ---- constants ----
        ident_bf = persist.tile([P, P], bf16)
        make_identity(nc, ident_bf[:])
        ones_f32 = persist.tile([P, 1], f32)
        nc.vector.memset(ones_f32[:], 1.0)
        one_f32 = persist.tile([1, 1], f32)
        nc.vector.memset(one_f32[:], 1.0)
        magic_u = persist.tile([P, 1], u32)
        nc.vector.memset(magic_u[:], MAGIC)

        # ---- persistent SBUF tensors ----
        mT = persist.tile([P, DC, S_SHARD], bf16)  # [d%128, d//128, s]
        m8 = persist.tile([P, ST, D], f8)  # [s%128, s//128, d] fp8
        xhatT = persist.tile([P, DC, B], bf16)  # [d%128, d//128, q]
        n2m = persist.tile([P, ST], f32)  # ||m_s||^2 / 64
        rs_m = persist.tile([P, ST], f32)  # 8 / ||m_s||
        rs_u = persist.tile([P, ST], u32)
        rs_t = persist.tile([P, ST], f32)
        xn2 = persist.tile([P, QT], f32)
        rs_x = persist.tile([P, QT], f32)
        xr_u = persist.tile([P, QT], u32)
        xr_t = persist.tile([P, QT], f32)

        def rsqrt_newton(dst, a, uscr, tscr, n, eng=None):
            """dst = 1/sqrt(a); all APs [P, n] f32 (uscr u32)."""
            if eng is None:
                eng = nc.vector
            mg = magic_u[:, 0:1]
            if n > 1:
                mg = mg.to_broadcast((P, n))
            eng.tensor_scalar(
                uscr, a.bitcast(u32), 1, None, ALU.logical_shift_right
            )
            eng.tensor_tensor(uscr, mg, uscr, ALU.subtract)
            y = uscr.bitcast(f32)
            for it in range(2):
                out_y = dst if it == 1 else y
                eng.tensor_tensor(tscr, y, y, ALU.mult)
                eng.tensor_tensor(tscr, tscr, a, ALU.mult)
                eng.tensor_scalar(tscr, tscr, -0.5, 1.5, ALU.mult, ALU.add)
                eng.tensor_tensor(out_y, y, tscr, ALU.mult)

        # ---- m load pipeline, split in two stages so the PE transposes are
        # emitted AFTER the current iteration's mm1 (they sit behind it in
        # the PE FIFO and never starve it).
        # stage a: batched DMA (MB tiles per dma_start), norms (ACT),
        #          bf16 + fp8 casts (DVE).  stage b: PE transpose + copy-back.
        # batch layout: first two batches are 2 tiles (so mT[0]'s data lands
        # ~3us sooner on the cold DMA queue), the rest 4
        _bat = [(0, 2), (2, 2)]
        while _bat[-1][0] + _bat[-1][1] < ST:
            _bat.append((_bat[-1][0] + _bat[-1][1], MB))
        _tile2bat = {}
        for bi, (b0, bn) in enumerate(_bat):
            for tt in range(b0, b0 + bn):
                _tile2bat[tt] = (bi, b0, bn)
        mf_batches = {}

        def load_stage_a(t):
            bi, b0, bn = _tile2bat[t]
            if bi not in mf_batches:
                mfb = loadp.tile([P, MB, D], f32, tag="mf", name=f"mfb_{bi}")
                nc.sync.dma_start(
                    out=mfb[:, 0:bn, :],
                    in_=m_ext[b0 * P : (b0 + bn) * P, :].rearrange(
                        "(j p) d -> p j d", p=P
                    ),
                )
                mf_batches[bi] = mfb
            mf = mf_batches[bi][:, t - b0, :]
            # fp8 out: the Square's tensor output is discarded (only
            # accum_out is used) -- writing it narrow trims ACT time
            msq = sqp.tile([P, D], f8, tag="sq", name=f"msq_{t}")
            # scale=1/8: accum collects sum((m/8)^2) = n2/64; rsqrt -> 8/||m||
            nc.scalar.activation(
                out=msq[:], in_=mf, func=AF.Square, scale=0.125,
                accum_out=n2m[:, t : t + 1],
            )
            if t % 8 == 7:
                sl = slice(t - 7, t + 1)
                rsqrt_newton(rs_m[:, sl], n2m[:, sl], rs_u[:, sl], rs_t[:, sl], 8)
            mb = mbp.tile([P, D], bf16, tag="mb", name=f"mb_{t}")
            nc.vector.tensor_copy(out=mb[:], in_=mf)
            nc.vector.tensor_copy(out=m8[:, t, :], in_=mb[:])
            return mb

        def load_stage_b(t, mb):
            mtp = psum_tp.tile([P, DC * P], bf16, tag="tp", name=f"mtp_{t}")
            for c in range(DC):
                nc.tensor.transpose(
                    mtp[:, c * P : (c + 1) * P],
                    mb[:, c * P : (c + 1) * P],
                    ident_bf[:],
                )
            nc.vector.tensor_copy(
                out=mT[:, :, t * P : (t + 1) * P],
                in_=mtp[:].rearrange("p (c q) -> p c q", c=DC),
            )

        loaded = set()

        def load_full(t):
            if t < ST and t not in loaded:
                loaded.add(t)
                load_stage_b(t, load_stage_a(t))

        # ---- x prep: per-tile DMA -> square -> rsqrt -> finish pipelines so
        # mm1(0) can start as soon as tile 3's chain completes (~8us) instead
        # of waiting on a monolithic half-DMA + batched rsqrt (~22us).
        # Half 1 (q 512:1024) is deferred into pass-0's load stream --
        # pass-0 never reads it, pass-1 starts ~100us later.
        xtiles = {}

        def x_dma(j):
            xh = xfp.tile([P, D], f32, tag=f"xf{j}", name=f"xfb_{j}")
            nc.sync.dma_start(
                out=xh[:],
                in_=x_ext[j * P : (j + 1) * P, :],
            )
            xtiles[j] = xh

        def xfb(j):
            return xtiles[j][:]

        def x_square(j):
            xsq = sqp.tile([P, D], f8, tag="sq", name=f"xsq_{j}")
            nc.scalar.activation(
                out=xsq[:], in_=xfb(j), func=AF.Square,
                accum_out=xn2[:, j : j + 1],
            )

        def x_rsqrt(j):
            rsqrt_newton(rs_x[:, j : j + 1], xn2[:, j : j + 1],
                         xr_u[:, j : j + 1], xr_t[:, j : j + 1], 1)

        def x_finish(j):
            xhat = work.tile([P, D], bf16, tag="xhat", name=f"xhat_{j}")
            nc.vector.tensor_scalar_mul(xhat[:], xfb(j), rs_x[:, j : j + 1])
            xtp = psum_tp.tile([P, DC * P], bf16, tag="tp", name=f"xtp_{j}")
            for c in range(DC):
                nc.tensor.transpose(
                    xtp[:, c * P : (c + 1) * P],
                    xhat[:, c * P : (c + 1) * P],
                    ident_bf[:],
                )
            nc.vector.tensor_copy(
                out=xhatT[:, :, j * P : (j + 1) * P],
                in_=xtp[:].rearrange("p (c q) -> p c q", c=DC),
            )

        # half-0 x: issue all 4 tile DMAs, then per-tile chains (emitted
        # BEFORE the m-prime so they lead every engine FIFO; mm1(0) needs
        # xhatT complete)
        for j in range(QT // 2):
            x_dma(j)
        for j in range(QT // 2):
            x_square(j)
            x_rsqrt(j)
            x_finish(j)

        # prime the m pipeline: DMA/norm/cast for LAG tiles, PE-transpose
        # only tile 0 now; the rest drain into pass-0's loop (2 per
        # iteration) so they sit BEHIND mm1 in the PE FIFO instead of
        # serializing before mm1(0) on the cold clock
        prime_b = []
        for u in range(LAG):
            loaded.add(u)
            mb_u = load_stage_a(u)
            if u == 0:
                load_stage_b(u, mb_u)
            else:
                prime_b.append((u, mb_u))

        # deferred half-1 thunks, consumed one per pass-0 iteration
        x_thunks = [lambda j=j: x_dma(j) for j in range(QT // 2, QT)]
        x_thunks.extend(
            [lambda j=j: x_square(j) for j in range(QT // 2, QT)]
        )
        x_thunks.append(lambda: rsqrt_newton(
            rs_x[:, QT // 2 :], xn2[:, QT // 2 :],
            xr_u[:, QT // 2 :], xr_t[:, QT // 2 :], QT // 2))
        x_thunks.extend([lambda j=j: x_finish(j) for j in range(QT // 2, QT)])

        # ---- main: scores^T -> exp(fp8) -> O (DoubleRow PSUM) / Z (DVE) ----
        rs_jobs = []
        for h, (q0, qp) in enumerate(PASSES):
            qpt = qp // P
            qr = qp // NCORES
            o2 = []
            for j in range(qpt):
                o2.append(psum_o.tile([P, D], f32, tag="o2", name=f"o2_{h}_{j}"))
            zacc = zp.tile([P, 2, QP], f32, tag="zacc", name=f"zacc_{h}")
            nc.gpsimd.memset(zacc[:], 0.0)

            def _mm1(t, mm2q, q0=q0, qp=qp, h=h):
                # mm2 single-matmul thunks are interleaved after chunks 0/2:
                # mm2's short fp8-DR stream (107ns) leaves its 154ns
                # LDWEIGHTS exposed when mm2s are back-to-back; tucked
                # between mm1's 213ns streams the LDW hides in the shadow
                # weight buffer
                sc = psum_sc.tile([P, QP], f32, tag="sc", name=f"sc_{h}_{t}")
                for c in range(DC):
                    nc.tensor.matmul(
                        sc[:, 0:qp],
                        mT[:, c, t * P : (t + 1) * P],
                        xhatT[:, c, q0 : q0 + qp],
                        start=(c == 0),
                        stop=(c == DC - 1),
                    )
                    if c in (0, 2) and mm2q:
                        mm2q.pop(0)()
                return sc

            def _exp(t, sc, pair, qp=qp):
                nc.scalar.activation(
                    out=pair[:, t % 2, 0:qp], in_=sc[:, 0:qp], func=AF.Exp,
                    scale=rs_m[:, t : t + 1],
                )

            def _mm2_one(u, pair, j, o2=o2):
                nc.tensor.matmul(
                    o2[j][:],
                    pair[:, :, j * P : (j + 1) * P],
                    m8[:, 2 * u : 2 * u + 2, :],
                    start=(u == 0),
                    stop=(u == NPAIR - 1),
                    perf_mode=DR,
                )

            def _mm2_ops(u, pair, qpt=qpt):
                return [
                    (lambda u=u, pair=pair, j=j: _mm2_one(u, pair, j))
                    for j in range(qpt)
                ]

            def _zadd(u, pair, zacc=zacc, qp=qp):
                # one GpSimd op accumulates the whole [P, 2, qp] pair (GpSimd
                # is otherwise idle; DVE is co-bottleneck in pass 0); the two
                # halves (even/odd s-tile) are summed later by the two
                # accumulating zsum matmuls
                nc.gpsimd.tensor_tensor(
                    zacc[:, :, 0:qp],
                    zacc[:, :, 0:qp],
                    pair[:, :, 0:qp],
                    ALU.add,
                )

            pairs = {}
            pending_b = list(prime_b) if h == 0 else []
            prime_b = []
            mm2q = []
            for t in range(ST):
                if h == 0:
                    ta = t + LAG
                    if ta < ST and ta not in loaded:
                        loaded.add(ta)
                        mb = load_stage_a(ta)
                        pending_b.append((ta, mb))
                u = t // 2
                if t >= 2 and t % 2 == 0:
                    mm2q.extend(_mm2_ops(u - 1, pairs.pop(u - 1)))
                sc = _mm1(t, mm2q)
                if t % 2 == 0:
                    pairs[u] = pt8p.tile(
                        [P, 2, QP], f8, tag="pt8", name=f"pt8_{h}_{u}"
                    )
                _exp(t, sc, pairs[u])
                if h == 0 and pending_b:
                    load_stage_b(*pending_b.pop(0))
                if h == 0 and len(pending_b) > 1:
                    load_stage_b(*pending_b.pop(0))
                if h == 0 and x_thunks and t >= 12:
                    # deferred past the DMA-bound warmup window (early PE
                    # gaps live in t<12); 13 thunks drain by t=25, well
                    # before pass 1 needs x half-1
                    x_thunks.pop(0)()
                if t % 2 == 1:
                    _zadd(u, pairs[u])
            # cross-partition Z reduce: [1, qp] = ones^T @ (zacc[0] + zacc[1]);
            # emitted BEFORE the final mm2 drain so the zsum->zrow->ztp
            # latency chain overlaps the trailing mm2 matmuls
            zsum = psum_tp.tile([1, QP], f32, tag="tp", name=f"zsum_{h}")
            nc.tensor.matmul(
                zsum[:, 0:qp], ones_f32[:], zacc[:, 0, 0:qp], start=True, stop=False
            )
            nc.tensor.matmul(
                zsum[:, 0:qp], ones_f32[:], zacc[:, 1, 0:qp], start=False, stop=True
            )
            zrow = finp.tile([1, QP], f32, tag="zrow", name=f"zrow_{h}")
            nc.vector.tensor_copy(out=zrow[0:1, 0:qp], in_=zsum[0:1, 0:qp])

            for op in _mm2_ops(NPAIR - 1, pairs.pop(NPAIR - 1)):
                op()

            ztp = psum_tp.tile([P, QPT], f32, tag="tp", name=f"ztp_{h}")
            for j in range(qpt):
                nc.tensor.transpose(
                    ztp[:, j : j + 1], zrow[0:1, j * P : (j + 1) * P], one_f32[:]
                )

            # stage [128, qpt, D+1] bf16: cols 0..D-1 = O, col D = Z
            stage = stp.tile([P, QPT, D + 1], bf16, tag="stage", name=f"stage_{h}")
            for j in range(qpt):
                nc.vector.tensor_copy(out=stage[:, j, 0:D], in_=o2[j][:])
            nc.vector.tensor_copy(
                out=stage[:, 0:qpt, D : D + 1],
                in_=ztp[:, 0:qpt].rearrange("p (j o) -> p j o", o=1),
            )
            partial = dram.tile(
                [qp, D + 1], bf16, tag=f"partial{qp}",
                name=f"partial_{h}", bufs=2,
            )
            nc.sync.dma_start(
                out=partial[:].rearrange("(o p) d -> p o d", p=P),
                in_=stage[:, 0:qpt, :],
            )
            # single A2A per pass: under the PE keep-alive the collective's
            # ~5us fixed cost dominates, so chunking the last pass loses
            chunks = [(q0, qp)]
            ro = 0
            for cq0, cqp in chunks:
                a2a = dram.tile(
                    [cqp, D + 1], bf16, tag=f"a2a{cqp}",
                    name=f"a2a_{h}_{cq0}", bufs=1,
                )
                nc.gpsimd.collective_compute(
                    "AllToAll",
                    mybir.AluOpType.bypass,
                    replica_groups=[list(range(NCORES))],
                    ins=[partial[ro : ro + cqp].opt()],
                    outs=[a2a[:].opt()],
                )
                rs_jobs.append(("a2a", cq0, cqp, a2a))
                ro += cqp

        # PE keep-alive: the HAM drops the core to a 50% utilization cap as
        # soon as engine activity drains, which halves the speed of the
        # trailing A2A + divide chain.  A stream of dependency-free dummy
        # matmuls holds the activity state up through the tail; a second,
        # fin8-gated block (emitted in the post loop below) tracks the
        # actual A2A completion so throttle-stretched collectives stay
        # covered through the divide chain.
        dummy_ps = psum_tp.tile([P, DC * P], bf16, tag="tp", name="dummy_ps")
        for k in range(150):
            nc.tensor.matmul(
                dummy_ps[:], ident_bf[:], mT[:, 0, 0 : DC * P], is_transpose=True
            )

        # post-RS divide + output: pushed to the end of the scheduler's
        # simulated timeline (tile_wait_until) so the strict-FIFO engine
        # queues can never stall mid-kernel on a collective dependency
        for kind, q0, qp, buf in rs_jobs:
            qr = qp // NCORES
            ctx2 = tc.tile_wait_until(5.0)
            ctx2.__enter__()
            if kind == "a2a":
                # a2a rows [qr*c : qr*(c+1)] = core c's partial for MY query
                # slice; sum the 8 blocks locally
                fin8 = finp.tile(
                    [QP // NCORES, NCORES, D + 1], bf16, tag="fin8",
                    name=f"fin8_{q0}", bufs=1,
                )[0:qr]
                nc.sync.dma_start(
                    out=fin8[:],
                    in_=buf[:].rearrange("(c p) d -> p c d", p=qr),
                )
                if q0 == PASSES[-1][0]:
                    # gated keep-alive: these wait for the final fin8 DMA
                    # (i.e. the last A2A), then hold PE activity through the
                    # divide chain however late the collective lands
                    for k in range(45):
                        nc.tensor.matmul(
                            dummy_ps[0:qr, 0 : D // 2],
                            ident_bf[0:qr, 0:qr],
                            fin8[:, k % NCORES, 0 : D // 2],
                            is_transpose=True,
                        )
                acc = finp.tile(
                    [QP // NCORES, D + 1], f32, tag="acc", name=f"acc_{q0}",
                    bufs=1,
                )[0:qr]
                nc.vector.tensor_tensor(
                    acc[:], fin8[:, 0, :], fin8[:, 1, :], ALU.add
                )
                for c in range(2, NCORES):
                    nc.vector.tensor_tensor(
                        acc[:], acc[:], fin8[:, c, :], ALU.add
                    )
                rz = finp.tile(
                    [QP // NCORES, 1], f32, tag="rz", name=f"rz_{q0}"
                )[0:qr]
                nc.vector.reciprocal(rz[:], acc[:, D : D + 1])
                outb = finp.tile(
                    [QP // NCORES, D], f32, tag="outb", name=f"outb_{q0}",
                    bufs=1,
                )[0:qr]
                nc.vector.tensor_scalar_mul(outb[:], acc[:, 0:D], rz[:])
                nc.sync.dma_start(
                    out=out_ext[q0 // NCORES : q0 // NCORES + qr, :],
                    in_=outb[:],
                )
            ctx2.__exit__(None, None, None)

    nc.compile()
    return nc


def _get_nc():
    if "nc" not in _CACHE:
        _CACHE["nc"] = _build()
    return _CACHE["nc"]


def _run(x, memory_bank, trace=False, **trace_kwargs):
    from concourse.bass_utils import run_bass_kernel_spmd

    nc = _get_nc()
    x = np.ascontiguousarray(np.asarray(x, dtype=np.float32))
    memory_bank = np.ascontiguousarray(np.asarray(memory_bank, dtype=np.float32))
    in_maps = [
        {
            "x": x,
            "mem": np.ascontiguousarray(
                memory_bank[i * S_SHARD : (i + 1) * S_SHARD]
            ),
        }
        for i in range(NCORES)
    ]
    res = run_bass_kernel_spmd(
        nc, in_maps, list(range(NCORES)), trace=trace, **trace_kwargs
    )
    # core i's output rows q0/8..q0/8+qr hold global q rows q0 + i*qr + k
    out = np.empty((B, D), dtype=np.float32)
    for i in range(NCORES):
        r = np.asarray(res.results[i]["out"])
        for q0, qp in CHUNKS:
            qr = qp // NCORES
            out[q0 + i * qr : q0 + (i + 1) * qr] = r[
                q0 // NCORES : q0 // NCORES + qr
            ]
    return out, res


def kernel(x, memory_bank):
    out, _ = _run(x, memory_bank)
    return out


if __name__ == "__main__":
    xs = np.random.randn(B, D).astype(np.float32)
    ms = np.random.randn(S, D).astype(np.float32)
    o = kernel(xs, ms)
    print(o.shape, o.dtype)

